# Initial kernel scaffold
#
"""Trainium2 Bass kernel for the CfGCN controller (gated K-hop graph-conv RNN).

Sharding: pure data parallel — batch B=64 split as 8 samples per NeuronCore,
processed on-chip as 4 pairs (2 samples stacked on the 128 partitions).
Weights replicated. Host does layout-only prep (transpose/reshape/pack).

Math (per sample, per step t):
    x_t   = frames[:, t]^T @ enc_W                    (encoder, N=64 nodes)
    q     = [h | x_t] @ Wq          -> q0|q1|q2       (packed gate weights)
    gates = q0 + A (q1 + A q2)                        (Horner in S^k)
    ff,gg,cc = split(gates); sig = sigmoid(ff)
    h     = tanh(cc) + sig*(tanh(gg) - tanh(cc))
    pooled_t = mean_n h                               (decoder deferred to end)
Decoder MLP runs once over all pooled vectors at the end.
"""

import os
from contextlib import ExitStack

import numpy as np

import concourse.bass as bass
import concourse.tile as tile
from concourse import bacc, mybir
from concourse.bass_utils import run_bass_kernel_spmd

B, T, C, N, D, OUT = 64, 128, 128, 64, 64, 6
NCORES = 8
BL = B // NCORES          # samples per core
NPAIR = BL // 2           # sample pairs per core
TW = 16                   # T-window (steps per DMA chunk)
F32 = mybir.dt.float32
BF16 = mybir.dt.bfloat16

# data dtype for the big streamed tensors + matmul operands
DATA_DT = BF16 if os.environ.get("KERNEL_DTYPE", "bf16") == "bf16" else F32


def _build(gate_bias_nonzero: bool):
    nc = bacc.Bacc("TRN2", target_bir_lowering=False, debug=False)
    dt = DATA_DT

    # ---- DRAM I/O (per-core shapes) ----
    fr_d = nc.dram_tensor("fr", [C, T, BL * N], dt, kind="ExternalInput")
    adjT_d = nc.dram_tensor("adjT", [128, T, NPAIR, N], dt, kind="ExternalInput")
    h0_d = nc.dram_tensor("h0", [128, NPAIR, D], F32, kind="ExternalInput")
    wq_d = nc.dram_tensor("wq", [128, 9 * D], dt, kind="ExternalInput")
    encw_d = nc.dram_tensor("encw", [C, D], dt, kind="ExternalInput")
    encb_d = nc.dram_tensor("encb", [D, 1], F32, kind="ExternalInput")
    gbias_d = nc.dram_tensor("gbias", [128, 3 * D], F32, kind="ExternalInput")
    ones_d = nc.dram_tensor("onesb", [128, 2], dt, kind="ExternalInput")
    ident_d = nc.dram_tensor("ident", [128, 128], dt, kind="ExternalInput")
    dw1_d = nc.dram_tensor("dw1", [D, 128], F32, kind="ExternalInput")
    db1_d = nc.dram_tensor("db1", [128, 1], F32, kind="ExternalInput")
    dw2_d = nc.dram_tensor("dw2", [128, D], F32, kind="ExternalInput")
    db2_d = nc.dram_tensor("db2", [D, 1], F32, kind="ExternalInput")
    dw3_d = nc.dram_tensor("dw3", [D, OUT], F32, kind="ExternalInput")
    dsb_d = nc.dram_tensor("dsb", [OUT, 2], F32, kind="ExternalInput")  # col0=scale, col1=bias

    ctrl_d = nc.dram_tensor("ctrl", [OUT, T * BL], F32, kind="ExternalOutput")
    hfin_d = nc.dram_tensor("hfin", [128, NPAIR, D], F32, kind="ExternalOutput")

    with TileCtx(nc) as tc, ExitStack() as ctx:
        const = ctx.enter_context(tc.tile_pool(name="const", bufs=1))
        win = ctx.enter_context(tc.tile_pool(name="win", bufs=2))
        state = ctx.enter_context(tc.tile_pool(name="state", bufs=2))
        work = ctx.enter_context(tc.tile_pool(name="work", bufs=3))
        stage = ctx.enter_context(tc.tile_pool(name="stage", bufs=2))
        nlin = ctx.enter_context(tc.tile_pool(name="nlin", bufs=2))
        poolbuf = ctx.enter_context(tc.tile_pool(name="poolbuf", bufs=1))
        psT = ctx.enter_context(tc.tile_pool(name="psT", bufs=1, space="PSUM"))
        psP = ctx.enter_context(tc.tile_pool(name="psP", bufs=2, space="PSUM"))
        psG = ctx.enter_context(tc.tile_pool(name="psG", bufs=1, space="PSUM"))
        psPool = ctx.enter_context(tc.tile_pool(name="psPool", bufs=1, space="PSUM"))
        psD = ctx.enter_context(tc.tile_pool(name="psD", bufs=2, space="PSUM"))

        # ---- constants ----
        wq = const.tile([128, 9 * D], dt)
        nc.sync.dma_start(wq[:], wq_d[:])
        encw = const.tile([C, D], dt)
        nc.sync.dma_start(encw[:], encw_d[:])
        encb = const.tile([D, 1], F32)
        nc.sync.dma_start(encb[:], encb_d[:])
        onesb = const.tile([128, 2], dt)
        nc.sync.dma_start(onesb[:], ones_d[:])
        ident = const.tile([128, 128], dt)
        nc.sync.dma_start(ident[:], ident_d[:])
        if gate_bias_nonzero:
            gbias = const.tile([128, 3 * D], F32)
            nc.sync.dma_start(gbias[:], gbias_d[:])

        # gates PSUM: one bank per pair (accumulation groups must not share banks)
        gates_ps = psG.tile([128, NPAIR, 512], F32)
        pool_ps = None
        pooled_sb = poolbuf.tile([D, T * BL], F32)

        # initial hidden state (natural layout [n-pair-stack, (pair, d)])
        h_nat = state.tile([128, NPAIR, D], F32)
        nc.sync.dma_start(h_nat[:], h0_d[:])

        for w in range(T // TW):
            fr_w = win.tile([C, TW, BL * N], dt, tag="fr_w")
            nc.sync.dma_start(fr_w[:], fr_d[:, w * TW:(w + 1) * TW, :])
            adj_w = win.tile([128, TW, NPAIR, N], dt, tag="adj_w")
            nc.sync.dma_start(adj_w[:], adjT_d[:, w * TW:(w + 1) * TW, :, :])

            for tl in range(TW):
                t = w * TW + tl

                if t % 64 == 0:
                    pool_ps = psPool.tile([D, 512], F32)

                # ---- hT (PE transpose) + xT (encoder) into one PSUM bank ----
                hx_ps = psT.tile([128, BL * N], F32)
                for g in range(NPAIR):
                    nc.tensor.transpose(
                        hx_ps[0:64, g * 128:(g + 1) * 128],
                        h_nat[:, g, :], ident[:],
                    )
                nc.tensor.matmul(
                    hx_ps[64:128, :], encw[:], fr_w[:, tl, :],
                    start=True, stop=True, tile_position=(0, 64),
                )
                hxt = work.tile([128, BL * N], dt, tag="hxt")
                # encoder bias folds into the PSUM->SBUF copy (per-partition AP)
                nc.vector.tensor_copy(hxt[0:64, :], hx_ps[0:64, :])
                if gate_bias_nonzero:
                    nc.scalar.activation(
                        hxt[64:128, :], hx_ps[64:128, :],
                        mybir.ActivationFunctionType.Identity, bias=encb[:],
                    )
                else:
                    nc.scalar.activation(
                        hxt[64:128, :], hx_ps[64:128, :],
                        mybir.ActivationFunctionType.Identity, bias=encb[:],
                    )

                for g in range(NPAIR):
                    at = adj_w[:, tl, g, :]            # [128, 64] A^T blocks
                    hslc = hxt[0:64, g * 128:(g + 1) * 128]
                    xslc = hxt[64:128, g * 128:(g + 1) * 128]
                    gps = gates_ps[:, g, 0:192]

                    # q0 (k=0 block) -> gates bank; h-part rows 0:64, x-part 64:128
                    nc.tensor.matmul(gps, hslc, wq[0:64, 0:192],
                                     start=True, stop=False)
                    nc.tensor.matmul(gps, xslc, wq[64:128, 0:192],
                                     start=False, stop=False)
                    # q1|q2 (k=1,2 block)
                    qp = psP.tile([128, 384], F32, tag="qp")
                    nc.tensor.matmul(qp[:], hslc, wq[0:64, 192:576],
                                     start=True, stop=False)
                    nc.tensor.matmul(qp[:], xslc, wq[64:128, 192:576],
                                     start=False, stop=False)
                    sq = stage.tile([128, 384], dt, tag="sq")
                    nc.vector.tensor_copy(sq[:], qp[:])
                    # t1 = q1 + A q2 (per-sample quadrant matmuls)
                    nc.tensor.matmul(qp[0:64, 0:192], at[0:64, :],
                                     sq[0:64, 192:384], start=False, stop=False)
                    nc.tensor.matmul(qp[64:128, 0:192], at[64:128, :],
                                     sq[64:128, 192:384], start=False, stop=True,
                                     tile_position=(64, 64))
                    st1 = stage.tile([128, 192], dt, tag="st1")
                    nc.scalar.copy(st1[:], qp[:, 0:192])
                    # gates += A t1
                    nc.tensor.matmul(gps[0:64, :], at[0:64, :], st1[0:64, :],
                                     start=False, stop=False)
                    nc.tensor.matmul(gps[64:128, :], at[64:128, :], st1[64:128, :],
                                     start=False, stop=True,
                                     tile_position=(64, 64))

                # ---- nonlinearities, batched across pairs ----
                gv = gates_ps[:]
                if gate_bias_nonzero:
                    nc.vector.tensor_add(
                        gv[:, :, 0:192], gv[:, :, 0:192],
                        gbias[:].broadcast(1, NPAIR),
                    )
                sig = nlin.tile([128, NPAIR, 64], F32, tag="sig")
                nc.scalar.activation(sig[:], gv[:, :, 0:64],
                                     mybir.ActivationFunctionType.Sigmoid)
                th = nlin.tile([128, NPAIR, 2, 64], F32, tag="th")
                nc.scalar.activation(th[:], gv[:, :, 64:192].rearrange2(NPAIR, 2, 64),
                                     mybir.ActivationFunctionType.Tanh)

                h_nat = state.tile([128, NPAIR, D], F32, tag="h_nat")
                tmp = work.tile([128, NPAIR, D], F32, tag="tmp")
                nc.vector.tensor_sub(tmp[:], th[:, :, 0, :], th[:, :, 1, :])
                nc.vector.tensor_mul(tmp[:], tmp[:], sig[:])
                nc.vector.tensor_add(h_nat[:], tmp[:], th[:, :, 1, :])

                # ---- node-mean pooling (tiny matmuls into persistent bank) ----
                for g in range(NPAIR):
                    nc.tensor.matmul(
                        pool_ps[:, (t % 64) * BL + 2 * g:(t % 64) * BL + 2 * g + 2],
                        h_nat[:, g, :], onesb[:],
                        start=True, stop=True,
                    )
                if t % 64 == 63:
                    nc.vector.tensor_copy(
                        pooled_sb[:, (t - 63) * BL:(t + 1) * BL], pool_ps[:],
                    )

        # ---- final hidden out ----
        hfin_sb = work.tile([128, NPAIR, D], F32, tag="hfin")
        nc.vector.tensor_copy(hfin_sb[:], h_nat[:])
        nc.sync.dma_start(hfin_d[:], hfin_sb[:])

        # ---- deferred decoder MLP over all pooled vectors ----
        dw1 = const.tile([D, 128], F32)
        nc.sync.dma_start(dw1[:], dw1_d[:])
        db1 = const.tile([128, 1], F32)
        nc.sync.dma_start(db1[:], db1_d[:])
        dw2 = const.tile([128, D], F32)
        nc.sync.dma_start(dw2[:], dw2_d[:])
        db2 = const.tile([D, 1], F32)
        nc.sync.dma_start(db2[:], db2_d[:])
        dw3 = const.tile([D, OUT], F32)
        nc.sync.dma_start(dw3[:], dw3_d[:])
        dsb = const.tile([OUT, 2], F32)
        nc.sync.dma_start(dsb[:], dsb_d[:])

        ctrl_sb = poolbuf.tile([OUT, T * BL], F32)
        for ch in range((T * BL) // 512):
            sl = slice(ch * 512, (ch + 1) * 512)
            z1p = psD.tile([128, 512], F32, tag="z1p")
            nc.tensor.matmul(z1p[:], dw1[:], pooled_sb[:, sl],
                             start=True, stop=True)
            z1 = work.tile([128, 512], F32, tag="z1")
            nc.scalar.activation(z1[:], z1p[:],
                                 mybir.ActivationFunctionType.Relu, bias=db1[:])
            z2p = psD.tile([D, 512], F32, tag="z2p")
            nc.tensor.matmul(z2p[:], dw2[:], z1[:], start=True, stop=True)
            z2 = work.tile([D, 512], F32, tag="z2")
            nc.scalar.activation(z2[:], z2p[:],
                                 mybir.ActivationFunctionType.Relu, bias=db2[:])
            cp = psD.tile([OUT, 512], F32, tag="cp")
            nc.tensor.matmul(cp[:], dw3[:], z2[:], start=True, stop=True)
            nc.scalar.activation(ctrl_sb[:, sl], cp[:],
                                 mybir.ActivationFunctionType.Identity,
                                 bias=dsb[:, 1:2], scale=dsb[:, 0:1])
        nc.sync.dma_start(ctrl_d[:], ctrl_sb[:])

    nc.compile()
    return nc


# TileContext alias (import here so _build reads cleanly)
TileCtx = tile.TileContext

_nc_cache = {}


def _get_nc(gate_bias_nonzero: bool):
    key = (gate_bias_nonzero, DATA_DT)
    if key not in _nc_cache:
        _nc_cache[key] = _build(gate_bias_nonzero)
    return _nc_cache[key]


def _np_dt(dt):
    import ml_dtypes
    return np.dtype(ml_dtypes.bfloat16) if dt == BF16 else np.dtype(np.float32)


def kernel(frames, adjacency, hidden_state, enc_W, enc_b,
           Wf_h, Wf_u, bf, Wg_h, Wg_u, bg, Wc_h, Wc_u, bc,
           dec_W1, dec_b1, dec_W2, dec_b2, dec_W3, dec_b3,
           out_scale, out_bias):
    frames = np.asarray(frames, np.float32).reshape(B, T, C, N)
    adjacency = np.asarray(adjacency, np.float32)
    hidden_state = np.asarray(hidden_state, np.float32)
    ddt = _np_dt(DATA_DT)

    # packed gate weight [128, 576]: rows 0:64 h-side, 64:128 x-side; f=(k,br,e)
    Wh = np.stack([Wf_h, Wg_h, Wc_h], axis=1).transpose(2, 0, 1, 3).reshape(D, 9 * D)
    Wu = np.stack([Wf_u, Wg_u, Wc_u], axis=1).transpose(2, 0, 1, 3).reshape(D, 9 * D)
    wq = np.ascontiguousarray(np.concatenate([Wh, Wu], 0), dtype=ddt)

    gb = np.concatenate([np.asarray(bf), np.asarray(bg), np.asarray(bc)])
    gate_bias_nonzero = bool(np.any(gb != 0))
    gbias = np.broadcast_to(gb.astype(np.float32), (128, 3 * D)).copy()

    nc = _get_nc(gate_bias_nonzero)

    onesb = np.zeros((128, 2), np.float32)
    onesb[0:64, 0] = 1.0 / N
    onesb[64:128, 1] = 1.0 / N
    ident = np.eye(128, dtype=np.float32)
    dsb = np.stack([np.asarray(out_scale, np.float32),
                    np.asarray(out_bias, np.float32)
                    + np.asarray(dec_b3, np.float32)
                    * np.asarray(out_scale, np.float32)], axis=1)

    common = {
        "wq": wq,
        "encw": np.ascontiguousarray(enc_W, dtype=ddt),
        "encb": np.asarray(enc_b, np.float32).reshape(D, 1),
        "gbias": gbias,
        "onesb": onesb.astype(ddt),
        "ident": ident.astype(ddt),
        "dw1": np.asarray(dec_W1, np.float32),
        "db1": np.asarray(dec_b1, np.float32).reshape(128, 1),
        "dw2": np.asarray(dec_W2, np.float32),
        "db2": np.asarray(dec_b2, np.float32).reshape(D, 1),
        "dw3": np.asarray(dec_W3, np.float32),
        "dsb": dsb,
    }

    in_maps = []
    for c in range(NCORES):
        s0 = c * BL
        fl = frames[s0:s0 + BL]                       # [8,T,C,N]
        fr = np.ascontiguousarray(
            fl.transpose(2, 1, 0, 3).reshape(C, T, BL * N), dtype=ddt)
        al = adjacency[s0:s0 + BL]                    # [8,T,N,N]
        # A^T with pair samples stacked on partitions: [128, T, NPAIR, N]
        adjT = np.ascontiguousarray(
            al.reshape(NPAIR, 2, T, N, N).transpose(1, 4, 2, 0, 3)
              .reshape(128, T, NPAIR, N), dtype=ddt)
        h0 = np.ascontiguousarray(
            hidden_state[s0:s0 + BL, 0]
            .reshape(NPAIR, 2, N, D).transpose(1, 2, 0, 3).reshape(128, NPAIR, D),
            dtype=np.float32)
        in_maps.append({"fr": fr, "adjT": adjT, "h0": h0, **common})

    res = run_bass_kernel_spmd(nc, in_maps, list(range(NCORES)))

    controls = np.zeros((B, T, OUT), np.float32)
    final_hidden = np.zeros((B, 1, N, D), np.float32)
    for c in range(NCORES):
        r = res.results[c]
        ctrl = r["ctrl"].reshape(OUT, T, BL).transpose(2, 1, 0)   # [8,T,6]
        controls[c * BL:(c + 1) * BL] = ctrl
        hf = r["hfin"].reshape(2, N, NPAIR, D).transpose(2, 0, 1, 3).reshape(BL, N, D)
        final_hidden[c * BL:(c + 1) * BL, 0] = hf
    return controls, final_hidden


# revision 11
# speedup vs baseline: 1.0138x; 1.0138x over previous
"""Trainium2 Bass kernel for the CfGCN controller (gated K-hop graph-conv RNN).

Sharding: pure data parallel — batch B=64 split as 8 samples per NeuronCore,
processed on-chip as 4 pairs (2 samples stacked on the 128 partitions).
Weights replicated. Host does layout-only prep (transpose/reshape/pack/cast).

Per sample, per step t:
    x_t   = frames[:, t]^T @ enc_W + enc_b            (encoder)
    q     = [h | x_t] @ Wq  -> q0|q1|q2               (packed gate weights)
    gates = q0 + A (q1 + A q2) + bias                 (Horner in S^k)
    ff,gg,cc = split(gates); sig = sigmoid(ff)
    h     = tanh(cc) + sig*(tanh(gg) - tanh(cc))
    pooled_t = mean_n h        (reduced from next step's h^T tile on DVE)
Decoder MLP over all pooled_t runs once at the end (batched over T);
the 1/64 node-mean scale is folded into dec_W1 on the host.
"""

import os
from contextlib import ExitStack

import numpy as np

import concourse.bass as bass  # noqa: F401
import concourse.tile as tile
from concourse import bacc, mybir
from concourse.bass_utils import run_bass_kernel_spmd

B, T, C, N, D, OUT = 64, 128, 128, 64, 64, 6
NCORES = 8
BL = B // NCORES          # samples per core
NPAIR = BL // 2           # sample pairs per core
TW = 16                   # steps per DMA window
F32 = mybir.dt.float32
BF16 = mybir.dt.bfloat16

DATA_DT = BF16 if os.environ.get("KERNEL_DTYPE", "bf16") == "bf16" else F32
AF = mybir.ActivationFunctionType
AX = mybir.AxisListType
ALU = mybir.AluOpType


def build(gate_bias_nonzero: bool, enc_bias_nonzero: bool, t_steps: int = T):
    nc = bacc.Bacc("TRN2", target_bir_lowering=False, debug=False)
    dt = DATA_DT

    fr_d = nc.dram_tensor("fr", [C, t_steps, BL * N], dt, kind="ExternalInput")
    # block-diag A^T pairs: [.., 0:64] cols sample0 / [.., 64:128] sample1
    adjT_d = nc.dram_tensor("adjT", [128, t_steps, NPAIR, 128], dt,
                            kind="ExternalInput")
    h0_d = nc.dram_tensor("h0", [128, NPAIR, D], dt, kind="ExternalInput")
    wq_d = nc.dram_tensor("wq", [128, 9 * D], dt, kind="ExternalInput")
    encw_d = nc.dram_tensor("encw", [C, D], dt, kind="ExternalInput")
    encb_d = nc.dram_tensor("encb", [128, 1], F32, kind="ExternalInput")
    gbias_d = nc.dram_tensor("gbias", [128, NPAIR, 3 * D], F32,
                             kind="ExternalInput")
    ident_d = nc.dram_tensor("ident", [128, 128], dt, kind="ExternalInput")
    dw1_d = nc.dram_tensor("dw1", [D, 128], F32, kind="ExternalInput")
    db1_d = nc.dram_tensor("db1", [128, 1], F32, kind="ExternalInput")
    dw2_d = nc.dram_tensor("dw2", [128, D], F32, kind="ExternalInput")
    db2_d = nc.dram_tensor("db2", [D, 1], F32, kind="ExternalInput")
    dw3_d = nc.dram_tensor("dw3", [D, OUT], F32, kind="ExternalInput")
    dsb_d = nc.dram_tensor("dsb", [OUT, 2], F32, kind="ExternalInput")

    ctrl_d = nc.dram_tensor("ctrl", [OUT, t_steps * BL], F32,
                            kind="ExternalOutput")
    hfin_d = nc.dram_tensor("hfin", [128, NPAIR, D], F32, kind="ExternalOutput")

    with tile.TileContext(nc) as tc, ExitStack() as ctx:
        const = ctx.enter_context(tc.tile_pool(name="const", bufs=1))
        win = ctx.enter_context(tc.tile_pool(name="win", bufs=2))
        state = ctx.enter_context(tc.tile_pool(name="state", bufs=2))
        work = ctx.enter_context(tc.tile_pool(name="work", bufs=3))
        stage = ctx.enter_context(tc.tile_pool(name="stage", bufs=3))
        nlin = ctx.enter_context(tc.tile_pool(name="nlin", bufs=2))
        big = ctx.enter_context(tc.tile_pool(name="big", bufs=1))
        psT = ctx.enter_context(tc.tile_pool(name="psT", bufs=1, space="PSUM"))
        psP = ctx.enter_context(tc.tile_pool(name="psP", bufs=2, space="PSUM"))
        psG = ctx.enter_context(tc.tile_pool(name="psG", bufs=1, space="PSUM"))

        wq = const.tile([128, 9 * D], dt, tag="wq")
        nc.sync.dma_start(wq[:], wq_d[:])
        encw = const.tile([C, D], dt, tag="encw")
        nc.sync.dma_start(encw[:], encw_d[:])
        ident = const.tile([128, 128], dt, tag="ident")
        nc.sync.dma_start(ident[:], ident_d[:])
        if enc_bias_nonzero:
            encb = const.tile([128, 1], F32, tag="encb")
            nc.sync.dma_start(encb[:], encb_d[:])
        if gate_bias_nonzero:
            gbias = const.tile([128, NPAIR, 3 * D], F32, tag="gbias")
            nc.sync.dma_start(gbias[:], gbias_d[:])

        # one PSUM bank per pair: accumulation groups must not share banks
        gates_ps = psG.tile([128, NPAIR, 512], F32, tag="gates")
        pooled_sb = big.tile([D, t_steps * BL], F32, tag="pooled")

        h_nat = state.tile([128, NPAIR, D], dt, tag="h_nat")
        nc.sync.dma_start(h_nat[:], h0_d[:])

        def build_hxt(t, with_x):
            """h^T (bf16 PE transposes) + x^T (fp32 encoder matmul) in
            separate PSUM banks, copied into one SBUF tile."""
            ht_ps = psT.tile([64, BL * N], dt, tag="ht")
            for g in range(NPAIR):
                nc.tensor.transpose(
                    ht_ps[:, g * 128:(g + 1) * 128],
                    h_nat[:, g, :], ident[:])
            if with_x:
                w, tl = divmod(t, TW)
                x_ps = psT.tile([128, BL * N], F32, tag="xe")
                nc.tensor.matmul(
                    x_ps[64:128, :], encw[:], fr_tiles[w][:, tl, :],
                    start=True, stop=True, tile_position=(0, 64))
            hxt = work.tile([128, BL * N], dt, tag="hxt")
            nc.vector.tensor_copy(hxt[0:64, :], ht_ps[:])
            if with_x:
                if enc_bias_nonzero:
                    nc.scalar.activation(hxt[64:128, :], x_ps[64:128, :],
                                         AF.Identity, bias=encb[64:128, :])
                else:
                    nc.scalar.copy(hxt[64:128, :], x_ps[64:128, :])
            return hxt

        def pool_from_hxt(hxt, tidx):
            """pooled[tidx] (node-sums of h^T halves) -> pooled_sb cols."""
            for g in range(NPAIR):
                nc.vector.tensor_reduce(
                    pooled_sb[:, tidx * BL + 2 * g:tidx * BL + 2 * g + 2],
                    hxt[0:64, g * 128:(g + 1) * 128].rearrange(
                        "p (s n) -> p s n", s=2),
                    AX.X, ALU.add)

        fr_tiles = {}
        for w in range(t_steps // TW):
            fr_tiles[w] = win.tile([C, TW, BL * N], dt, name="fr_w", tag="fr_w")
            nc.sync.dma_start(fr_tiles[w][:], fr_d[:, w * TW:(w + 1) * TW, :])
            adj_w = win.tile([128, TW, NPAIR, 128], dt, tag="adj_w")
            nc.sync.dma_start(adj_w[:], adjT_d[:, w * TW:(w + 1) * TW, :, :])

            for tl in range(TW):
                t = w * TW + tl
                hxt = build_hxt(t, with_x=True)
                if t > 0:
                    pool_from_hxt(hxt, t - 1)

                for g in range(NPAIR):
                    at = adj_w[:, tl, g, :]        # [128,128] A^T block-diag
                    hxslc = hxt[:, g * 128:(g + 1) * 128]
                    gps = gates_ps[:, g, 0:192]

                    nc.tensor.matmul(gps, hxslc, wq[:, 0:192],
                                     start=True, stop=True)
                    qp = psP.tile([128, 384], F32, tag="qp")
                    nc.tensor.matmul(qp[:], hxslc, wq[:, 192:576],
                                     start=True, stop=True)
                    # stage only q2 to SBUF; q1 stays in PSUM and accumulates
                    sq = stage.tile([128, 192], dt, tag="sq")
                    nc.vector.tensor_copy(sq[:], qp[:, 192:384])
                    # q1 += A q2
                    nc.tensor.matmul(qp[:, 0:192], at, sq[:],
                                     start=False, stop=False,
                                     skip_group_check=True)
                    st1 = stage.tile([128, 192], dt, tag="st1")
                    if g % 2 == 0:
                        nc.scalar.copy(st1[:], qp[:, 0:192])
                    else:
                        nc.vector.tensor_copy(st1[:], qp[:, 0:192])
                    # gates += A (q1 + A q2)
                    nc.tensor.matmul(gps, at, st1[:],
                                     start=False, stop=False,
                                     skip_group_check=True)

                gv = gates_ps[:]
                if gate_bias_nonzero:
                    nc.vector.tensor_add(gv[:, :, 0:192], gv[:, :, 0:192],
                                         gbias[:])
                sig = nlin.tile([128, NPAIR, 64], F32, tag="sig")
                nc.scalar.activation(sig[:], gv[:, :, 0:64], AF.Sigmoid)
                th = nlin.tile([128, NPAIR, 128], F32, tag="th")
                nc.scalar.activation(th[:], gv[:, :, 64:192], AF.Tanh)

                h_nat = state.tile([128, NPAIR, D], dt, tag="h_nat")
                tmp = work.tile([128, NPAIR, D], F32, tag="tmp")
                nc.gpsimd.tensor_sub(tmp[:], th[:, :, 0:64], th[:, :, 64:128])
                nc.gpsimd.tensor_mul(tmp[:], tmp[:], sig[:])
                nc.vector.tensor_add(h_nat[:], tmp[:], th[:, :, 64:128])

        # final-step pooling needs one more h^T build (no encoder half)
        hxt = build_hxt(t_steps, with_x=False)
        pool_from_hxt(hxt, t_steps - 1)

        hfin_sb = work.tile([128, NPAIR, D], F32, tag="hfin")
        nc.vector.tensor_copy(hfin_sb[:], h_nat[:])
        nc.sync.dma_start(hfin_d[:], hfin_sb[:])

        # deferred decoder MLP over all pooled vectors
        dw1 = const.tile([D, 128], F32, tag="dw1")
        nc.sync.dma_start(dw1[:], dw1_d[:])
        db1 = const.tile([128, 1], F32, tag="db1")
        nc.sync.dma_start(db1[:], db1_d[:])
        dw2 = const.tile([128, D], F32, tag="dw2")
        nc.sync.dma_start(dw2[:], dw2_d[:])
        db2 = const.tile([D, 1], F32, tag="db2")
        nc.sync.dma_start(db2[:], db2_d[:])
        dw3 = const.tile([D, OUT], F32, tag="dw3")
        nc.sync.dma_start(dw3[:], dw3_d[:])
        dsb = const.tile([OUT, 2], F32, tag="dsb")
        nc.sync.dma_start(dsb[:], dsb_d[:])

        ctrl_sb = big.tile([OUT, t_steps * BL], F32, tag="ctrl")
        nchunk = max(1, (t_steps * BL) // 512)
        csz = (t_steps * BL) // nchunk
        for ch in range(nchunk):
            sl = slice(ch * csz, (ch + 1) * csz)
            z1p = psP.tile([128, csz], F32, tag="qp")
            nc.tensor.matmul(z1p[:], dw1[:], pooled_sb[:, sl],
                             start=True, stop=True)
            z1 = work.tile([128, csz], F32, tag="z1")
            nc.scalar.activation(z1[:], z1p[:], AF.Relu, bias=db1[:])
            z2p = psP.tile([D, csz], F32, tag="qp")
            nc.tensor.matmul(z2p[:], dw2[:], z1[:], start=True, stop=True)
            z2 = work.tile([D, csz], F32, tag="z2")
            nc.scalar.activation(z2[:], z2p[:], AF.Relu, bias=db2[:])
            cp = psP.tile([OUT, csz], F32, tag="qp")
            nc.tensor.matmul(cp[:], dw3[:], z2[:], start=True, stop=True)
            nc.scalar.activation(ctrl_sb[:, sl], cp[:], AF.Identity,
                                 bias=dsb[:, 1:2], scale=dsb[:, 0:1])
        nc.sync.dma_start(ctrl_d[:], ctrl_sb[:])

    nc.compile()
    return nc


_nc_cache = {}


def _get_nc(gate_bias_nonzero, enc_bias_nonzero):
    key = (gate_bias_nonzero, enc_bias_nonzero, DATA_DT)
    if key not in _nc_cache:
        _nc_cache[key] = build(gate_bias_nonzero, enc_bias_nonzero)
    return _nc_cache[key]


def _np_dt():
    import ml_dtypes
    return np.dtype(ml_dtypes.bfloat16) if DATA_DT == BF16 else np.dtype(np.float32)


def prep_inputs(frames, adjacency, hidden_state, enc_W, enc_b,
                Wf_h, Wf_u, bf, Wg_h, Wg_u, bg, Wc_h, Wc_u, bc,
                dec_W1, dec_b1, dec_W2, dec_b2, dec_W3, dec_b3,
                out_scale, out_bias, t_steps=T):
    """Host-side layout prep. Returns (in_maps, gate_bias_nonzero, enc_bias_nonzero)."""
    ddt = _np_dt()
    frames = np.asarray(frames, np.float32).reshape(B, -1, C, N)[:, :t_steps]
    adjacency = np.asarray(adjacency, np.float32)[:, :t_steps]
    hidden_state = np.asarray(hidden_state, np.float32)

    Wh = np.stack([Wf_h, Wg_h, Wc_h], axis=1).transpose(2, 0, 1, 3).reshape(D, 9 * D)
    Wu = np.stack([Wf_u, Wg_u, Wc_u], axis=1).transpose(2, 0, 1, 3).reshape(D, 9 * D)
    wq = np.ascontiguousarray(np.concatenate([Wh, Wu], 0), dtype=ddt)

    gb = np.concatenate([np.asarray(bf), np.asarray(bg), np.asarray(bc)])
    gate_bias_nonzero = bool(np.any(gb != 0))
    gbias = np.ascontiguousarray(np.broadcast_to(
        gb.astype(np.float32), (128, NPAIR, 3 * D)))
    enc_bias_nonzero = bool(np.any(np.asarray(enc_b) != 0))
    encb = np.zeros((128, 1), np.float32)
    encb[64:128, 0] = np.asarray(enc_b, np.float32)

    dsb = np.stack([np.asarray(out_scale, np.float32),
                    np.asarray(out_bias, np.float32)
                    + np.asarray(dec_b3, np.float32)
                    * np.asarray(out_scale, np.float32)], axis=1)

    common = {
        "wq": wq,
        "encw": np.ascontiguousarray(enc_W, dtype=ddt),
        "encb": encb,
        "gbias": gbias,
        "ident": np.eye(128, dtype=ddt),
        # node-mean 1/64 folded into dec_W1 (pooled_sb holds node sums)
        "dw1": np.ascontiguousarray(np.asarray(dec_W1, np.float32) / N),
        "db1": np.asarray(dec_b1, np.float32).reshape(128, 1),
        "dw2": np.ascontiguousarray(dec_W2, np.float32),
        "db2": np.asarray(dec_b2, np.float32).reshape(D, 1),
        "dw3": np.ascontiguousarray(dec_W3, np.float32),
        "dsb": np.ascontiguousarray(dsb),
    }

    in_maps = []
    for c in range(NCORES):
        s0 = c * BL
        fl = frames[s0:s0 + BL]                       # [8,Ts,C,N]
        fr = np.ascontiguousarray(
            fl.transpose(2, 1, 0, 3).reshape(C, t_steps, BL * N), dtype=ddt)
        al = adjacency[s0:s0 + BL]                    # [8,Ts,N,N]
        # block-diag A^T pairs: adjT[64*q+m, t, g, 64*q'+n] =
        #   A[2g+q, t, n, m] if q == q' else 0
        aT = al.reshape(NPAIR, 2, t_steps, N, N).transpose(1, 4, 2, 0, 3)
        adjT = np.zeros((2, N, t_steps, NPAIR, 2, N), np.float32)
        adjT[0, :, :, :, 0] = aT[0]
        adjT[1, :, :, :, 1] = aT[1]
        adjT = np.ascontiguousarray(
            adjT.reshape(128, t_steps, NPAIR, 128), dtype=ddt)
        h0 = np.ascontiguousarray(
            hidden_state[s0:s0 + BL, 0]
            .reshape(NPAIR, 2, N, D).transpose(1, 2, 0, 3).reshape(128, NPAIR, D),
            dtype=ddt)
        in_maps.append({"fr": fr, "adjT": adjT, "h0": h0, **common})
    return in_maps, gate_bias_nonzero, enc_bias_nonzero


def unshard_outputs(results, t_steps=T):
    controls = np.zeros((B, t_steps, OUT), np.float32)
    final_hidden = np.zeros((B, 1, N, D), np.float32)
    for c in range(NCORES):
        r = results[c]
        ctrl = np.asarray(r["ctrl"], np.float32).reshape(OUT, t_steps, BL)
        controls[c * BL:(c + 1) * BL] = ctrl.transpose(2, 1, 0)
        hf = np.asarray(r["hfin"], np.float32).reshape(2, N, NPAIR, D)
        final_hidden[c * BL:(c + 1) * BL, 0] = (
            hf.transpose(2, 0, 1, 3).reshape(BL, N, D))
    return controls, final_hidden


def kernel(**inputs):
    in_maps, gbnz, ebnz = prep_inputs(**inputs)
    nc = _get_nc(gbnz, ebnz)
    res = run_bass_kernel_spmd(nc, in_maps, list(range(NCORES)))
    return unshard_outputs(res.results)


# revision 12
# speedup vs baseline: 1.0889x; 1.0741x over previous
"""Trainium2 Bass kernel for the CfGCN controller (gated K-hop graph-conv RNN).

Sharding: pure data parallel — batch B=64 split as 8 samples per NeuronCore,
processed on-chip as 4 pairs (2 samples stacked on the 128 partitions).
Weights replicated. Host does layout-only prep (transpose/reshape/pack/cast).

Per sample, per step t:
    x_t   = frames[:, t]^T @ enc_W + enc_b            (encoder)
    q     = [h | x_t] @ Wq  -> q0|q1|q2               (packed gate weights)
    gates = q0 + A (q1 + A q2) + bias                 (Horner in S^k)
    ff,gg,cc = split(gates); sig = sigmoid(ff)
    h     = tanh(cc) + sig*(tanh(gg) - tanh(cc))
    pooled_t = mean_n h        (reduced from next step's h^T tile on DVE)
Decoder MLP over all pooled_t runs once at the end (batched over T);
the 1/64 node-mean scale is folded into dec_W1 on the host.
"""

import os
from contextlib import ExitStack

import numpy as np

import concourse.bass as bass  # noqa: F401
import concourse.tile as tile
from concourse import bacc, mybir
from concourse.bass_utils import run_bass_kernel_spmd

B, T, C, N, D, OUT = 64, 128, 128, 64, 64, 6
NCORES = 8
BL = B // NCORES          # samples per core
NPAIR = BL // 2           # sample pairs per core
TW = 16                   # steps per DMA window
F32 = mybir.dt.float32
BF16 = mybir.dt.bfloat16

DATA_DT = BF16 if os.environ.get("KERNEL_DTYPE", "bf16") == "bf16" else F32
AF = mybir.ActivationFunctionType
AX = mybir.AxisListType
ALU = mybir.AluOpType


def build(gate_bias_nonzero: bool, enc_bias_nonzero: bool, t_steps: int = T):
    nc = bacc.Bacc("TRN2", target_bir_lowering=False, debug=False)
    dt = DATA_DT

    fr_d = nc.dram_tensor("fr", [C, t_steps, BL * N], dt, kind="ExternalInput")
    # block-diag A^T pairs: [.., 0:64] cols sample0 / [.., 64:128] sample1
    adjT_d = nc.dram_tensor("adjT", [128, t_steps, NPAIR, 128], dt,
                            kind="ExternalInput")
    h0_d = nc.dram_tensor("h0", [128, NPAIR, D], dt, kind="ExternalInput")
    wq_d = nc.dram_tensor("wq", [128, 9 * D], dt, kind="ExternalInput")
    encw_d = nc.dram_tensor("encw", [C, D], dt, kind="ExternalInput")
    encb_d = nc.dram_tensor("encb", [128, 1], F32, kind="ExternalInput")
    gbias_d = nc.dram_tensor("gbias", [128, NPAIR, 3 * D], F32,
                             kind="ExternalInput")
    ident_d = nc.dram_tensor("ident", [128, 128], dt, kind="ExternalInput")
    dw1_d = nc.dram_tensor("dw1", [D, 128], F32, kind="ExternalInput")
    db1_d = nc.dram_tensor("db1", [128, 1], F32, kind="ExternalInput")
    dw2_d = nc.dram_tensor("dw2", [128, D], F32, kind="ExternalInput")
    db2_d = nc.dram_tensor("db2", [D, 1], F32, kind="ExternalInput")
    dw3_d = nc.dram_tensor("dw3", [D, OUT], F32, kind="ExternalInput")
    dsb_d = nc.dram_tensor("dsb", [OUT, 2], F32, kind="ExternalInput")

    ctrl_d = nc.dram_tensor("ctrl", [OUT, t_steps * BL], F32,
                            kind="ExternalOutput")
    hfin_d = nc.dram_tensor("hfin", [128, NPAIR, D], F32, kind="ExternalOutput")

    with tile.TileContext(nc) as tc, ExitStack() as ctx:
        const = ctx.enter_context(tc.tile_pool(name="const", bufs=1))
        win = ctx.enter_context(tc.tile_pool(name="win", bufs=2))
        state = ctx.enter_context(tc.tile_pool(name="state", bufs=2))
        work = ctx.enter_context(tc.tile_pool(name="work", bufs=3))
        stage = ctx.enter_context(tc.tile_pool(name="stage", bufs=3))
        nlin = ctx.enter_context(tc.tile_pool(name="nlin", bufs=2))
        big = ctx.enter_context(tc.tile_pool(name="big", bufs=1))
        psT = ctx.enter_context(tc.tile_pool(name="psT", bufs=1, space="PSUM"))
        psP = ctx.enter_context(tc.tile_pool(name="psP", bufs=2, space="PSUM"))
        psG = ctx.enter_context(tc.tile_pool(name="psG", bufs=1, space="PSUM"))

        wq = const.tile([128, 9 * D], dt, tag="wq")
        nc.sync.dma_start(wq[:], wq_d[:])
        encw = const.tile([C, D], dt, tag="encw")
        nc.sync.dma_start(encw[:], encw_d[:])
        ident = const.tile([128, 128], dt, tag="ident")
        nc.sync.dma_start(ident[:], ident_d[:])
        if enc_bias_nonzero:
            encb = const.tile([128, 1], F32, tag="encb")
            nc.sync.dma_start(encb[:], encb_d[:])
        if gate_bias_nonzero:
            gbias = const.tile([128, NPAIR, 3 * D], F32, tag="gbias")
            nc.sync.dma_start(gbias[:], gbias_d[:])

        # one PSUM bank per pair: accumulation groups must not share banks
        gates_ps = psG.tile([128, NPAIR, 512], F32, tag="gates")
        pooled_sb = big.tile([D, t_steps * BL], F32, tag="pooled")

        h_nat = state.tile([128, NPAIR, D], dt, tag="h_nat")
        nc.sync.dma_start(h_nat[:], h0_d[:])

        def build_hxt(t, with_x):
            """h^T (bf16 PE transposes) + x^T (fp32 encoder matmul) in
            separate PSUM banks, copied into one SBUF tile."""
            ht_ps = psT.tile([64, BL * N], dt, tag="ht")
            for g in range(NPAIR):
                nc.tensor.transpose(
                    ht_ps[:, g * 128:(g + 1) * 128],
                    h_nat[:, g, :], ident[:])
            if with_x:
                w, tl = divmod(t, TW)
                x_ps = psT.tile([128, BL * N], F32, tag="xe")
                nc.tensor.matmul(
                    x_ps[64:128, :], encw[:], fr_tiles[w][:, tl, :],
                    start=True, stop=True, tile_position=(0, 64))
            hxt = work.tile([128, BL * N], dt, tag="hxt")
            nc.vector.tensor_copy(hxt[0:64, :], ht_ps[:])
            if with_x:
                if enc_bias_nonzero:
                    nc.scalar.activation(hxt[64:128, :], x_ps[64:128, :],
                                         AF.Identity, bias=encb[64:128, :])
                else:
                    nc.scalar.copy(hxt[64:128, :], x_ps[64:128, :])
            return hxt

        def pool_from_hxt(hxt, tidx):
            """pooled[tidx] (node-sums of h^T halves) -> pooled_sb cols."""
            nc.vector.tensor_reduce(
                pooled_sb[:, tidx * BL:(tidx + 1) * BL],
                hxt[0:64, :].rearrange("p (s n) -> p s n", s=BL),
                AX.X, ALU.add)

        fr_tiles = {}
        for w in range(t_steps // TW):
            fr_tiles[w] = win.tile([C, TW, BL * N], dt, name="fr_w", tag="fr_w")
            nc.sync.dma_start(fr_tiles[w][:], fr_d[:, w * TW:(w + 1) * TW, :])
            adj_w = win.tile([128, TW, NPAIR, 128], dt, tag="adj_w")
            nc.sync.dma_start(adj_w[:], adjT_d[:, w * TW:(w + 1) * TW, :, :])

            for tl in range(TW):
                t = w * TW + tl
                hxt = build_hxt(t, with_x=True)
                if t > 0:
                    pool_from_hxt(hxt, t - 1)

                for g in range(NPAIR):
                    at = adj_w[:, tl, g, :]        # [128,128] A^T block-diag
                    hxslc = hxt[:, g * 128:(g + 1) * 128]
                    gps = gates_ps[:, g, 0:192]

                    nc.tensor.matmul(gps, hxslc, wq[:, 0:192],
                                     start=True, stop=True)
                    qp = psP.tile([128, 384], F32, tag="qp")
                    nc.tensor.matmul(qp[:], hxslc, wq[:, 192:576],
                                     start=True, stop=True)
                    # stage only q2 to SBUF; q1 stays in PSUM and accumulates
                    sq = stage.tile([128, 192], dt, tag="sq")
                    nc.vector.tensor_copy(sq[:], qp[:, 192:384])
                    # q1 += A q2
                    nc.tensor.matmul(qp[:, 0:192], at, sq[:],
                                     start=False, stop=False,
                                     skip_group_check=True)
                    st1 = stage.tile([128, 192], dt, tag="st1")
                    if g % 2 == 0:
                        nc.scalar.copy(st1[:], qp[:, 0:192])
                    else:
                        nc.vector.tensor_copy(st1[:], qp[:, 0:192])
                    # gates += A (q1 + A q2)
                    nc.tensor.matmul(gps, at, st1[:],
                                     start=False, stop=False,
                                     skip_group_check=True)

                gv = gates_ps[:]
                if gate_bias_nonzero:
                    nc.vector.tensor_add(gv[:, :, 0:192], gv[:, :, 0:192],
                                         gbias[:])
                sig = nlin.tile([128, NPAIR, 64], F32, tag="sig")
                nc.scalar.activation(sig[:], gv[:, :, 0:64], AF.Sigmoid)
                th = nlin.tile([128, NPAIR, 128], F32, tag="th")
                nc.scalar.activation(th[:], gv[:, :, 64:192], AF.Tanh)

                h_nat = state.tile([128, NPAIR, D], dt, tag="h_nat")
                tmp = work.tile([128, NPAIR, D], F32, tag="tmp")
                nc.vector.tensor_sub(tmp[:], th[:, :, 0:64], th[:, :, 64:128])
                nc.vector.tensor_mul(tmp[:], tmp[:], sig[:])
                nc.vector.tensor_add(h_nat[:], tmp[:], th[:, :, 64:128])

        # final-step pooling needs one more h^T build (no encoder half)
        hxt = build_hxt(t_steps, with_x=False)
        pool_from_hxt(hxt, t_steps - 1)

        hfin_sb = work.tile([128, NPAIR, D], F32, tag="hfin")
        nc.vector.tensor_copy(hfin_sb[:], h_nat[:])
        nc.sync.dma_start(hfin_d[:], hfin_sb[:])

        # deferred decoder MLP over all pooled vectors
        dw1 = const.tile([D, 128], F32, tag="dw1")
        nc.sync.dma_start(dw1[:], dw1_d[:])
        db1 = const.tile([128, 1], F32, tag="db1")
        nc.sync.dma_start(db1[:], db1_d[:])
        dw2 = const.tile([128, D], F32, tag="dw2")
        nc.sync.dma_start(dw2[:], dw2_d[:])
        db2 = const.tile([D, 1], F32, tag="db2")
        nc.sync.dma_start(db2[:], db2_d[:])
        dw3 = const.tile([D, OUT], F32, tag="dw3")
        nc.sync.dma_start(dw3[:], dw3_d[:])
        dsb = const.tile([OUT, 2], F32, tag="dsb")
        nc.sync.dma_start(dsb[:], dsb_d[:])

        ctrl_sb = big.tile([OUT, t_steps * BL], F32, tag="ctrl")
        nchunk = max(1, (t_steps * BL) // 512)
        csz = (t_steps * BL) // nchunk
        for ch in range(nchunk):
            sl = slice(ch * csz, (ch + 1) * csz)
            z1p = psP.tile([128, csz], F32, tag="qp")
            nc.tensor.matmul(z1p[:], dw1[:], pooled_sb[:, sl],
                             start=True, stop=True)
            z1 = work.tile([128, csz], F32, tag="z1")
            nc.scalar.activation(z1[:], z1p[:], AF.Relu, bias=db1[:])
            z2p = psP.tile([D, csz], F32, tag="qp")
            nc.tensor.matmul(z2p[:], dw2[:], z1[:], start=True, stop=True)
            z2 = work.tile([D, csz], F32, tag="z2")
            nc.scalar.activation(z2[:], z2p[:], AF.Relu, bias=db2[:])
            cp = psP.tile([OUT, csz], F32, tag="qp")
            nc.tensor.matmul(cp[:], dw3[:], z2[:], start=True, stop=True)
            nc.scalar.activation(ctrl_sb[:, sl], cp[:], AF.Identity,
                                 bias=dsb[:, 1:2], scale=dsb[:, 0:1])
        nc.sync.dma_start(ctrl_d[:], ctrl_sb[:])

    nc.compile()
    return nc


_nc_cache = {}


def _get_nc(gate_bias_nonzero, enc_bias_nonzero):
    key = (gate_bias_nonzero, enc_bias_nonzero, DATA_DT)
    if key not in _nc_cache:
        _nc_cache[key] = build(gate_bias_nonzero, enc_bias_nonzero)
    return _nc_cache[key]


def _np_dt():
    import ml_dtypes
    return np.dtype(ml_dtypes.bfloat16) if DATA_DT == BF16 else np.dtype(np.float32)


def prep_inputs(frames, adjacency, hidden_state, enc_W, enc_b,
                Wf_h, Wf_u, bf, Wg_h, Wg_u, bg, Wc_h, Wc_u, bc,
                dec_W1, dec_b1, dec_W2, dec_b2, dec_W3, dec_b3,
                out_scale, out_bias, t_steps=T):
    """Host-side layout prep. Returns (in_maps, gate_bias_nonzero, enc_bias_nonzero)."""
    ddt = _np_dt()
    frames = np.asarray(frames, np.float32).reshape(B, -1, C, N)[:, :t_steps]
    adjacency = np.asarray(adjacency, np.float32)[:, :t_steps]
    hidden_state = np.asarray(hidden_state, np.float32)

    Wh = np.stack([Wf_h, Wg_h, Wc_h], axis=1).transpose(2, 0, 1, 3).reshape(D, 9 * D)
    Wu = np.stack([Wf_u, Wg_u, Wc_u], axis=1).transpose(2, 0, 1, 3).reshape(D, 9 * D)
    wq = np.ascontiguousarray(np.concatenate([Wh, Wu], 0), dtype=ddt)

    gb = np.concatenate([np.asarray(bf), np.asarray(bg), np.asarray(bc)])
    gate_bias_nonzero = bool(np.any(gb != 0))
    gbias = np.ascontiguousarray(np.broadcast_to(
        gb.astype(np.float32), (128, NPAIR, 3 * D)))
    enc_bias_nonzero = bool(np.any(np.asarray(enc_b) != 0))
    encb = np.zeros((128, 1), np.float32)
    encb[64:128, 0] = np.asarray(enc_b, np.float32)

    dsb = np.stack([np.asarray(out_scale, np.float32),
                    np.asarray(out_bias, np.float32)
                    + np.asarray(dec_b3, np.float32)
                    * np.asarray(out_scale, np.float32)], axis=1)

    common = {
        "wq": wq,
        "encw": np.ascontiguousarray(enc_W, dtype=ddt),
        "encb": encb,
        "gbias": gbias,
        "ident": np.eye(128, dtype=ddt),
        # node-mean 1/64 folded into dec_W1 (pooled_sb holds node sums)
        "dw1": np.ascontiguousarray(np.asarray(dec_W1, np.float32) / N),
        "db1": np.asarray(dec_b1, np.float32).reshape(128, 1),
        "dw2": np.ascontiguousarray(dec_W2, np.float32),
        "db2": np.asarray(dec_b2, np.float32).reshape(D, 1),
        "dw3": np.ascontiguousarray(dec_W3, np.float32),
        "dsb": np.ascontiguousarray(dsb),
    }

    in_maps = []
    for c in range(NCORES):
        s0 = c * BL
        fl = frames[s0:s0 + BL]                       # [8,Ts,C,N]
        fr = np.ascontiguousarray(
            fl.transpose(2, 1, 0, 3).reshape(C, t_steps, BL * N), dtype=ddt)
        al = adjacency[s0:s0 + BL]                    # [8,Ts,N,N]
        # block-diag A^T pairs: adjT[64*q+m, t, g, 64*q'+n] =
        #   A[2g+q, t, n, m] if q == q' else 0
        aT = al.reshape(NPAIR, 2, t_steps, N, N).transpose(1, 4, 2, 0, 3)
        adjT = np.zeros((2, N, t_steps, NPAIR, 2, N), np.float32)
        adjT[0, :, :, :, 0] = aT[0]
        adjT[1, :, :, :, 1] = aT[1]
        adjT = np.ascontiguousarray(
            adjT.reshape(128, t_steps, NPAIR, 128), dtype=ddt)
        h0 = np.ascontiguousarray(
            hidden_state[s0:s0 + BL, 0]
            .reshape(NPAIR, 2, N, D).transpose(1, 2, 0, 3).reshape(128, NPAIR, D),
            dtype=ddt)
        in_maps.append({"fr": fr, "adjT": adjT, "h0": h0, **common})
    return in_maps, gate_bias_nonzero, enc_bias_nonzero


def unshard_outputs(results, t_steps=T):
    controls = np.zeros((B, t_steps, OUT), np.float32)
    final_hidden = np.zeros((B, 1, N, D), np.float32)
    for c in range(NCORES):
        r = results[c]
        ctrl = np.asarray(r["ctrl"], np.float32).reshape(OUT, t_steps, BL)
        controls[c * BL:(c + 1) * BL] = ctrl.transpose(2, 1, 0)
        hf = np.asarray(r["hfin"], np.float32).reshape(2, N, NPAIR, D)
        final_hidden[c * BL:(c + 1) * BL, 0] = (
            hf.transpose(2, 0, 1, 3).reshape(BL, N, D))
    return controls, final_hidden


def kernel(**inputs):
    in_maps, gbnz, ebnz = prep_inputs(**inputs)
    nc = _get_nc(gbnz, ebnz)
    res = run_bass_kernel_spmd(nc, in_maps, list(range(NCORES)))
    return unshard_outputs(res.results)
